# revision 1
# baseline (speedup 1.0000x reference)
"""Trainium2 Bass kernel: single-head causal attention, data-parallel over batch.

Problem: x [4096, 64, 128] f32, Wq/Wk/Wv [128, 64] f32.
  q,k,v = x @ W*;  scores = q k^T / sqrt(128); causal softmax; out = attn @ v.

Sharding: batch 4096 -> 8 cores x 512 batches. Each core loops over 32
super-tiles of 16 batches (1024 rows of x).

Structure:
  * A-trick: scores = x A x^T with A = Wq Wk^T / sqrt(C) folded on host.
  * SWDGE cast-DMA loads x f32->bf16 straight from HBM.
  * Batch-pair packing: P_S computes [128,128] = diag(scores_e^T, scores_o^T)
    + off-diag garbage in ONE matmul (mask zeroes garbage after exp); P4 uses
    diag(E_e, E_o) as one stationary: [O'|sums] = E^T.T @ [V|ones] per pair.
  * SOFTWARE-PIPELINED EMISSION: engine queues are in-order, so scores/exp/
    mask/P4/normalize of tile X-1 are emitted interleaved into tile X's
    stream. The PE queue per iteration is [transposes(X), P_S(X-1), Y(X),
    v(X), P4(X-1)] -- every matmul's inputs are ready by the time the PE
    reaches it, instead of the PE stalling on the ACT/DVE softmax chain.

Per-core pipeline (bf16 matmuls, fp32 PSUM):
  1. SWDGE DMA-cast x tile [128, 1024] f32->bf16.
  2. 8 PE transposes -> x^T (PSUM bf16) -> SBUF (vector).
  3. Y = A x^T: 2 matmuls N=512 (stationary A^T) -> PSUM -> SBUF bf16 (scalar).
  4. v = x wv: 8 matmuls (stationary x^T blocks, rhs wv) -> SBUF + ones col.
  5. P_S: 8 pair-matmuls -> sc_ps [128, 1024] (diag-packed scores^T).
  6. exp on ACT (PSUM->SBUF bf16), mask-mult on vector (zeroes garbage).
  7. P4: 8 pair-matmuls [O' | sums].
  8. normalize: O = O' * recip(sums); DMA out.
"""

import os
import numpy as np
import ml_dtypes
from contextlib import ExitStack

B, T, C, H = 4096, 64, 128, 64
N_CORES = 8
ST_B = 16                    # batches per super-tile
ROWS = ST_B * T              # 1024
B_CORE = B // N_CORES        # 512
N_ST = B_CORE // ST_B        # 32

_cached = {}


def _build_nc():
    import concourse.bass as bass
    import concourse.mybir as mybir
    import concourse.tile as tile
    from concourse import bacc

    F32 = mybir.dt.float32
    BF16 = mybir.dt.bfloat16

    nc = bacc.Bacc("TRN2", target_bir_lowering=False, debug=False)
    x_d = nc.dram_tensor("x", [B_CORE * T, C], F32, kind="ExternalInput").ap()
    at_d = nc.dram_tensor("at", [C, C], BF16, kind="ExternalInput").ap()
    wv_d = nc.dram_tensor("wv", [C, H], BF16, kind="ExternalInput").ap()
    id_d = nc.dram_tensor("ident", [C, C], BF16, kind="ExternalInput").ap()
    mk_d = nc.dram_tensor("mask", [128, 1024], BF16, kind="ExternalInput").ap()
    o_d = nc.dram_tensor("o", [B_CORE * T, H], F32, kind="ExternalOutput").ap()

    with tile.TileContext(nc) as tc, ExitStack() as ctx:
        sb = ctx.enter_context(tc.tile_pool(name="sb", bufs=4))
        ps = ctx.enter_context(tc.tile_pool(name="ps", bufs=1, space="PSUM"))
        psO = ctx.enter_context(tc.tile_pool(name="psO", bufs=1, space="PSUM"))
        cpool = ctx.enter_context(tc.tile_pool(name="const", bufs=1))

        at_sb = cpool.tile([C, C], BF16, tag="at")
        wv_sb = cpool.tile([C, H], BF16, tag="wv")
        id_sb = cpool.tile([C, C], BF16, tag="id")
        mk_sb = cpool.tile([128, 1024], BF16, tag="mk")
        nc.sync.dma_start(at_sb[:], at_d)
        nc.sync.dma_start(wv_sb[:], wv_d)
        nc.sync.dma_start(id_sb[:], id_d)
        nc.sync.dma_start(mk_sb[:], mk_d)

        xv = x_d.rearrange("(S n p) c -> S p n c", n=8, p=128)
        ov = o_d.rearrange("(S m par t) h -> S (par t) m h", m=8, par=2, t=64)

        def emit_scores(p):
            """P_S + exp + mask for tile p (emitted one iteration late)."""
            sc_ps = ps.tile([128, 1024], F32, tag="sc", name="sc_ps")
            for m in range(8):
                nc.tensor.matmul(
                    sc_ps[:, 128 * m:128 * m + 128],
                    p["y_sb"][:, 128 * m:128 * m + 128],
                    p["xT_sb"][:, 128 * m:128 * m + 128],
                    start=True, stop=True,
                )
            E_raw = sb.tile([128, 1024], BF16, tag="Eraw", name="E_raw")
            nc.scalar.activation(
                E_raw[:], sc_ps[:], mybir.ActivationFunctionType.Exp
            )
            E_sb = sb.tile([128, 1024], BF16, tag="E", name="E_sb")
            nc.vector.tensor_tensor(
                out=E_sb[:], in0=E_raw[:], in1=mk_sb[:],
                op=mybir.AluOpType.mult,
            )
            p["E_sb"] = E_sb

        def emit_tail(p):
            """P4 + normalize + store for tile p (emitted one iteration late)."""
            o_ps = psO.tile([128, 1024], F32, tag="o", name="o_ps")
            for m in range(8):
                off = 512 * (m // 4) + 65 * (m % 4)
                nc.tensor.matmul(
                    o_ps[:, off:off + 65],
                    p["E_sb"][:, 128 * m:128 * m + 128],
                    p["v_sb"][:, 66 * m:66 * m + 65],
                    start=True, stop=True,
                )
            opsv = o_ps[:].rearrange("p (B x) -> p B x", B=2)[:, :, 0:260]
            opsb = opsv.rearrange("p B (m z) -> p B m z", z=65)
            r_sb = sb.tile([128, 8], F32, tag="r", name="r_sb")
            r_v = r_sb[:].rearrange("p (B m) -> p B m", B=2)
            nc.vector.reciprocal(r_v.unsqueeze(3), opsb[:, :, :, 64:65])
            o_sb = sb.tile([128, 512], F32, tag="o_sb", name="o_sb")
            nc.vector.tensor_tensor(
                out=o_sb[:].rearrange("p (B m t) -> p B m t", B=2, t=64),
                in0=opsb[:, :, :, 0:64],
                in1=r_v.unsqueeze(3).broadcast_to((128, 2, 4, 64)),
                op=mybir.AluOpType.mult,
            )
            nc.sync.dma_start(
                ov[p["st"]], o_sb[:].rearrange("p (m h) -> p m h", h=64))

        prev = None
        for st in range(N_ST):
            cur = {"st": st}

            # ---- SWDGE cast-load x (f32 HBM -> bf16 SBUF)
            x_bf = sb.tile([128, ROWS], BF16, tag="x_bf")
            nc.gpsimd.dma_start(
                x_bf[:].rearrange("p (n c) -> p n c", n=8), xv[st]
            )

            # ---- 8 PE transposes -> xT in PSUM (bf16), then copy to SBUF
            xT_ps = ps.tile([128, ROWS // 2], F32, tag="xT")
            xT_ps_bf = xT_ps[:].bitcast(BF16)
            for i in range(8):
                nc.tensor.transpose(
                    xT_ps_bf[:, 128 * i:128 * (i + 1)],
                    x_bf[:, 128 * i:128 * (i + 1)],
                    id_sb[:],
                )
            xT_sb = sb.tile([128, ROWS], BF16, tag="xT_sb")
            nc.vector.tensor_copy(xT_sb[:], xT_ps_bf)
            cur["xT_sb"] = xT_sb

            # ---- deferred: scores/exp/mask of previous tile (fills the
            # PE queue while this tile's xT copy runs on vector)
            if prev is not None:
                emit_scores(prev)

            # ---- Y = A x^T (stationary A^T), 2 matmuls N=512
            y_ps = ps.tile([128, 1024], F32, tag="y")
            for half in range(2):
                nc.tensor.matmul(
                    y_ps[:, 512 * half:512 * half + 512],
                    at_sb[:],
                    xT_sb[:, 512 * half:512 * half + 512],
                    start=True, stop=True,
                )
            y_sb = sb.tile([128, 1024], BF16, tag="y_sb")
            nc.scalar.copy(y_sb[:], y_ps[:])
            cur["y_sb"] = y_sb

            # ---- v = x @ wv (stationary x^T blocks, rhs wv)
            v_ps = ps.tile([128, 512], F32, tag="v")
            for m in range(8):
                nc.tensor.matmul(
                    v_ps[:, 64 * m:64 * m + 64],
                    xT_sb[:, 128 * m:128 * m + 128],
                    wv_sb[:],
                    start=True, stop=True,
                )
            v_sb = sb.tile([128, 8 * 66], BF16, tag="v_sb")
            v_sb_v = v_sb[:].rearrange("p (m z) -> p m z", z=66)
            nc.vector.tensor_copy(
                v_sb_v[:, :, 0:64],
                v_ps[:].rearrange("p (m t) -> p m t", t=64),
            )
            nc.gpsimd.memset(v_sb_v[:, :, 64:65], 1.0)
            cur["v_sb"] = v_sb

            # ---- deferred: P4/normalize/store of previous tile
            if prev is not None:
                emit_tail(prev)

            prev = cur

        # drain the pipeline for the last tile
        emit_scores(prev)
        emit_tail(prev)

    nc.compile()
    return nc


def _host_inputs(x, Wq, Wk, Wv):
    bf = ml_dtypes.bfloat16
    at = np.ascontiguousarray((Wk @ Wq.T * (C ** -0.5)).astype(bf))
    wv_bf = np.ascontiguousarray(Wv.astype(bf))
    ident = np.eye(128, dtype=bf)
    tri = np.triu(np.ones((T, T), dtype=np.float32))  # [s, t]: 1 if s <= t
    mask_pair = np.kron(np.eye(2, dtype=np.float32), tri)  # [128, 128]
    mask = np.ascontiguousarray(np.tile(mask_pair, (1, 8)).astype(bf))
    in_maps = []
    for c in range(N_CORES):
        shard = np.ascontiguousarray(
            x[c * B_CORE:(c + 1) * B_CORE].reshape(B_CORE * T, C)
        ).astype(np.float32)
        in_maps.append({
            "x": shard, "at": at, "wv": wv_bf,
            "ident": ident, "mask": mask,
        })
    return in_maps


def run(x, Wq, Wk, Wv, trace=False, **run_kwargs):
    from concourse import bass_utils

    if "nc" not in _cached:
        _cached["nc"] = _build_nc()
    nc = _cached["nc"]
    in_maps = _host_inputs(np.asarray(x), np.asarray(Wq),
                           np.asarray(Wk), np.asarray(Wv))
    res = bass_utils.run_bass_kernel_spmd(
        nc, in_maps, core_ids=list(range(N_CORES)), trace=trace, **run_kwargs
    )
    outs = [r["o"].reshape(B_CORE, T, H) for r in res.results]
    return np.concatenate(outs, axis=0), res


def kernel(x, Wq, Wk, Wv):
    out, _ = run(x, Wq, Wk, Wv, trace=False)
    return out



# revision 2
# speedup vs baseline: 1.0684x; 1.0684x over previous
"""Trainium2 Bass kernel: single-head causal attention, data-parallel over batch.

Problem: x [4096, 64, 128] f32, Wq/Wk/Wv [128, 64] f32.
  q,k,v = x @ W*;  scores = q k^T / sqrt(128); causal softmax; out = attn @ v.

Sharding: batch 4096 -> 8 cores x 512 batches. Each core loops over 32
super-tiles of 16 batches (1024 rows of x).

Host prep: x is cast to bf16 and pre-transposed to x^T [C=128, rows] per
core, so the device does plain contiguous HWDGE loads (2 KB/partition)
and needs no PE transposes. A = Wq Wk^T / sqrt(C) folded on host.

Per-core pipeline (bf16 matmuls, fp32 PSUM), 3-stage software pipeline
(iteration st emits stage A(st), B(st-1), C(st-2)) so the serial chain
P_S -> exp -> mask -> P4 spans two iterations and every engine-queue
entry has its inputs ready when the engine reaches it:

  A(st): prefetch-DMA x^T(st+1); Y = A x^T (2 MMs, stationary A^T);
         y copy PSUM->SBUF bf16 split ACT/DVE; v = x wv (8 MMs,
         stationary x^T blocks); v copy (DVE) + ones col (gpsimd).
  B(st-1): P_S: 8 pair-matmuls -> scores^T diag-packed [128, 1024];
         exp on ACT (PSUM->SBUF bf16); causal+pair mask mult on GPSIMD.
  C(st-2): P4: 8 pair-matmuls [O' | sums]; recip + normalize (DVE,
         bf16 out); contiguous store [128, 512] -> HBM (host un-permutes).
"""

import numpy as np
import ml_dtypes
from contextlib import ExitStack

B, T, C, H = 4096, 64, 128, 64
N_CORES = 8
ST_B = 16                    # batches per super-tile
ROWS = ST_B * T              # 1024
B_CORE = B // N_CORES        # 512
N_ST = B_CORE // ST_B        # 32
Y_ACT = 736                  # y-copy columns done on ACT (rest on DVE)

_cached = {}


def _build_nc():
    import concourse.bass as bass
    import concourse.mybir as mybir
    import concourse.tile as tile
    from concourse import bacc

    F32 = mybir.dt.float32
    BF16 = mybir.dt.bfloat16

    nc = bacc.Bacc("TRN2", target_bir_lowering=False, debug=False)
    x_d = nc.dram_tensor("xt", [C, B_CORE * T], BF16, kind="ExternalInput").ap()
    at_d = nc.dram_tensor("at", [C, C], BF16, kind="ExternalInput").ap()
    wv_d = nc.dram_tensor("wv", [C, H], BF16, kind="ExternalInput").ap()
    mk_d = nc.dram_tensor("mask", [128, 1024], BF16, kind="ExternalInput").ap()
    o_d = nc.dram_tensor("o", [128, N_ST * 512], BF16, kind="ExternalOutput").ap()

    with tile.TileContext(nc) as tc, ExitStack() as ctx:
        sb = ctx.enter_context(tc.tile_pool(name="sb", bufs=4))
        ps = ctx.enter_context(tc.tile_pool(name="ps", bufs=1, space="PSUM"))
        cpool = ctx.enter_context(tc.tile_pool(name="const", bufs=1))

        at_sb = cpool.tile([C, C], BF16, tag="at")
        wv_sb = cpool.tile([C, H], BF16, tag="wv")
        mk_sb = cpool.tile([128, 1024], BF16, tag="mk")
        nc.sync.dma_start(at_sb[:], at_d)
        nc.sync.dma_start(wv_sb[:], wv_d)
        nc.sync.dma_start(mk_sb[:], mk_d)

        xv = x_d.rearrange("p (S n) -> S p n", n=ROWS)
        ov = o_d.rearrange("p (S f) -> S p f", f=512)

        def emit_load(st):
            """HWDGE load x^T tile [128, 1024] bf16, 2KB/partition."""
            xT_sb = sb.tile([128, ROWS], BF16, tag="xT")
            nc.sync.dma_start(xT_sb[:], xv[st])
            return xT_sb

        def emit_A(st, xT_sb):
            """Y + y-copy + v + v-copy for tile st."""
            cur = {"st": st, "xT_sb": xT_sb}

            y_ps = ps.tile([128, 1024], F32, tag="y")
            for half in range(2):
                nc.tensor.matmul(
                    y_ps[:, 512 * half:512 * half + 512],
                    at_sb[:],
                    xT_sb[:, 512 * half:512 * half + 512],
                    start=True, stop=True,
                )
            y_sb = sb.tile([128, 1024], BF16, tag="y_sb")
            nc.scalar.copy(y_sb[:, 0:Y_ACT], y_ps[:, 0:Y_ACT])
            nc.vector.tensor_copy(y_sb[:, Y_ACT:1024], y_ps[:, Y_ACT:1024])
            cur["y_sb"] = y_sb

            v_ps = ps.tile([128, 512], F32, tag="v")
            for m in range(8):
                nc.tensor.matmul(
                    v_ps[:, 64 * m:64 * m + 64],
                    xT_sb[:, 128 * m:128 * m + 128],
                    wv_sb[:],
                    start=True, stop=True,
                )
            v_sb = sb.tile([128, 8 * 66], BF16, tag="v_sb")
            v_sb_v = v_sb[:].rearrange("p (m z) -> p m z", z=66)
            nc.vector.tensor_copy(
                v_sb_v[:, :, 0:64],
                v_ps[:].rearrange("p (m t) -> p m t", t=64),
            )
            nc.gpsimd.memset(v_sb_v[:, :, 64:65], 1.0)
            cur["v_sb"] = v_sb
            return cur

        def emit_B(p):
            """P_S + exp (ACT) + mask (GPSIMD) for tile p."""
            sc_ps = ps.tile([128, 1024], F32, tag="sc", name="sc_ps")
            for m in range(8):
                nc.tensor.matmul(
                    sc_ps[:, 128 * m:128 * m + 128],
                    p["y_sb"][:, 128 * m:128 * m + 128],
                    p["xT_sb"][:, 128 * m:128 * m + 128],
                    start=True, stop=True,
                )
            E_raw = sb.tile([128, 1024], BF16, tag="Eraw", name="E_raw")
            nc.scalar.activation(
                E_raw[:], sc_ps[:], mybir.ActivationFunctionType.Exp
            )
            E_sb = sb.tile([128, 1024], BF16, tag="E", name="E_sb")
            nc.gpsimd.tensor_tensor(
                out=E_sb[:], in0=E_raw[:], in1=mk_sb[:],
                op=mybir.AluOpType.mult,
            )
            p["E_sb"] = E_sb

        def emit_C(p):
            """P4 + normalize + store for tile p."""
            o_ps = ps.tile([128, 1024], F32, tag="o", name="o_ps")
            for m in range(8):
                off = 512 * (m // 4) + 65 * (m % 4)
                nc.tensor.matmul(
                    o_ps[:, off:off + 65],
                    p["E_sb"][:, 128 * m:128 * m + 128],
                    p["v_sb"][:, 66 * m:66 * m + 65],
                    start=True, stop=True,
                )
            opsv = o_ps[:].rearrange("p (B x) -> p B x", B=2)[:, :, 0:260]
            opsb = opsv.rearrange("p B (m z) -> p B m z", z=65)
            r_sb = sb.tile([128, 8], F32, tag="r", name="r_sb")
            r_v = r_sb[:].rearrange("p (B m) -> p B m", B=2)
            nc.vector.reciprocal(r_v.unsqueeze(3), opsb[:, :, :, 64:65])
            o_sb = sb.tile([128, 512], BF16, tag="o_sb", name="o_sb")
            nc.vector.tensor_tensor(
                out=o_sb[:].rearrange("p (B m t) -> p B m t", B=2, t=64),
                in0=opsb[:, :, :, 0:64],
                in1=r_v.unsqueeze(3).broadcast_to((128, 2, 4, 64)),
                op=mybir.AluOpType.mult,
            )
            nc.sync.dma_start(ov[p["st"]], o_sb[:])

        tiles = {}
        tiles[0] = {"xT_sb": emit_load(0)}
        for st in range(N_ST):
            if st + 1 < N_ST:
                tiles[st + 1] = {"xT_sb": emit_load(st + 1)}
            tiles[st] = emit_A(st, tiles[st]["xT_sb"])
            if st >= 1:
                emit_B(tiles[st - 1])
            if st >= 2:
                emit_C(tiles[st - 2])
        emit_B(tiles[N_ST - 1])
        emit_C(tiles[N_ST - 2])
        emit_C(tiles[N_ST - 1])

    nc.compile()
    return nc


def _host_inputs(x, Wq, Wk, Wv):
    bf = ml_dtypes.bfloat16
    at = np.ascontiguousarray((Wk @ Wq.T * (C ** -0.5)).astype(bf))
    wv_bf = np.ascontiguousarray(Wv.astype(bf))
    tri = np.triu(np.ones((T, T), dtype=np.float32))  # [s, t]: 1 if s <= t
    mask_pair = np.kron(np.eye(2, dtype=np.float32), tri)  # [128, 128]
    mask = np.ascontiguousarray(np.tile(mask_pair, (1, 8)).astype(bf))
    in_maps = []
    for c in range(N_CORES):
        shard = x[c * B_CORE:(c + 1) * B_CORE].reshape(B_CORE * T, C)
        xt = np.ascontiguousarray(shard.T).astype(bf)  # [128, 32768]
        in_maps.append({
            "xt": xt, "at": at, "wv": wv_bf, "mask": mask,
        })
    return in_maps


def _unshard(res_list):
    outs = []
    for r in res_list:
        o = np.asarray(r["o"], dtype=np.float32)  # [128, N_ST*512]
        # o[par*64+t, st*512 + m*64 + h] -> out[(st, m, par), t, h]
        o = o.reshape(2, 64, N_ST, 8, 64).transpose(2, 3, 0, 1, 4)
        outs.append(np.ascontiguousarray(o.reshape(B_CORE, T, H)))
    return np.concatenate(outs, axis=0)


def run(x, Wq, Wk, Wv, trace=False, **run_kwargs):
    from concourse import bass_utils

    if "nc" not in _cached:
        _cached["nc"] = _build_nc()
    nc = _cached["nc"]
    in_maps = _host_inputs(np.asarray(x), np.asarray(Wq),
                           np.asarray(Wk), np.asarray(Wv))
    res = bass_utils.run_bass_kernel_spmd(
        nc, in_maps, core_ids=list(range(N_CORES)), trace=trace, **run_kwargs
    )
    return _unshard(res.results), res


def kernel(x, Wq, Wk, Wv):
    out, _ = run(x, Wq, Wk, Wv, trace=False)
    return out


# revision 3
# speedup vs baseline: 1.2771x; 1.1954x over previous
"""Trainium2 Bass kernel: single-head causal attention, data-parallel over batch.

Problem: x [4096, 64, 128] f32, Wq/Wk/Wv [128, 64] f32.
  q,k,v = x @ W*;  scores = q k^T / sqrt(128); causal softmax; out = attn @ v.

Sharding: batch 4096 -> 8 cores x 512 batches. Each core loops over 32
super-tiles of 16 batches (1024 rows of x).

Host prep: x cast to bf16 and pre-transposed to x^T [C=128, rows] per core
(plain contiguous HWDGE loads, no PE transposes); A = Wq Wk^T / sqrt(C)
folded on host.

Compact-S dataflow: P_S is COL-TILED on the PE array — per batch-pair, two
concurrent M=64 matmuls (tile_position (0,0) / (0,64)) emit only the two
valid diagonal 64x64 score blocks, stacked on partitions: sc [128, 512]
with NO cross-batch garbage. exp and causal mask process half the data of
the pair-packed variant. P4 is ROW+COL-TILED: per pair, two concurrent
K=64/M=64 matmuls (tile (0,0) and (64,64)) contract each batch's E block
with its v rows, yielding the same [t-pair, 65] output packing.

3-stage software pipeline (iteration st emits A(st), B(st-1), C(st-2)):
  A(st): paired prefetch DMA (2 tiles / 512 KB); Y = A x^T (2 MMs);
         y copy PSUM->SBUF bf16 on ACT; v = x wv (8 MMs); v copy (DVE)
         + ones col (gpsimd memset).
  B(st-1): P_S 16 col-tiled MMs -> sc [128, 512]; exp on ACT; causal
         mask mult on GPSIMD.
  C(st-2): P4 16 row+col-tiled MMs [O' | sums]; recip + normalize (DVE,
         bf16 out); contiguous store [128, 512] (host un-permutes).
"""

import numpy as np
import ml_dtypes
from contextlib import ExitStack

B, T, C, H = 4096, 64, 128, 64
N_CORES = 8
ST_B = 16                    # batches per super-tile
ROWS = ST_B * T              # 1024
B_CORE = B // N_CORES        # 512
N_ST = B_CORE // ST_B        # 32

_cached = {}


def _build_nc():
    import concourse.bass as bass
    import concourse.mybir as mybir
    import concourse.tile as tile
    from concourse import bacc

    F32 = mybir.dt.float32
    BF16 = mybir.dt.bfloat16

    nc = bacc.Bacc("TRN2", target_bir_lowering=False, debug=False)
    x_d = nc.dram_tensor("xt", [C, B_CORE * T], BF16, kind="ExternalInput").ap()
    at_d = nc.dram_tensor("at", [C, C], BF16, kind="ExternalInput").ap()
    wv_d = nc.dram_tensor("wv", [C, H], BF16, kind="ExternalInput").ap()
    mk_d = nc.dram_tensor("mask", [128, 512], BF16, kind="ExternalInput").ap()
    o_d = nc.dram_tensor("o", [128, N_ST * 512], BF16, kind="ExternalOutput").ap()

    with tile.TileContext(nc) as tc, ExitStack() as ctx:
        sb = ctx.enter_context(tc.tile_pool(name="sb", bufs=4))
        ps = ctx.enter_context(tc.tile_pool(name="ps", bufs=1, space="PSUM"))
        cpool = ctx.enter_context(tc.tile_pool(name="const", bufs=1))

        at_sb = cpool.tile([C, C], BF16, tag="at")
        wv_sb = cpool.tile([C, H], BF16, tag="wv")
        mk_sb = cpool.tile([128, 512], BF16, tag="mk")
        nc.sync.dma_start(at_sb[:], at_d)
        nc.sync.dma_start(wv_sb[:], wv_d)
        nc.sync.dma_start(mk_sb[:], mk_d)

        # paired loads: one DMA brings 2 super-tiles (512 KB, 4KB/partition)
        xv = x_d.rearrange("p (P n) -> P p n", n=2 * ROWS)
        ov = o_d.rearrange("p (S f) -> S p f", f=512)

        def emit_load(pair):
            xT2 = sb.tile([128, 2 * ROWS], BF16, tag="xT2")
            nc.sync.dma_start(xT2[:], xv[pair])
            return xT2

        def emit_A(st, xT_sb):
            """Y + y-copy + v + v-copy for tile st."""
            cur = {"st": st, "xT_sb": xT_sb}

            y_ps = ps.tile([128, 1024], F32, tag="y")
            for half in range(2):
                nc.tensor.matmul(
                    y_ps[:, 512 * half:512 * half + 512],
                    at_sb[:],
                    xT_sb[:, 512 * half:512 * half + 512],
                    start=True, stop=True,
                )
            y_sb = sb.tile([128, 1024], BF16, tag="y_sb")
            nc.scalar.copy(y_sb[:], y_ps[:])
            cur["y_sb"] = y_sb

            v_ps = ps.tile([128, 512], F32, tag="v")
            for m in range(8):
                nc.tensor.matmul(
                    v_ps[:, 64 * m:64 * m + 64],
                    xT_sb[:, 128 * m:128 * m + 128],
                    wv_sb[:],
                    start=True, stop=True,
                )
            v_sb = sb.tile([128, 8 * 66], BF16, tag="v_sb")
            v_sb_v = v_sb[:].rearrange("p (m z) -> p m z", z=66)
            nc.vector.tensor_copy(
                v_sb_v[:, :, 0:64],
                v_ps[:].rearrange("p (m t) -> p m t", t=64),
            )
            nc.gpsimd.memset(v_sb_v[:, :, 64:65], 1.0)
            cur["v_sb"] = v_sb
            return cur

        def emit_B(p):
            """Col-tiled P_S + exp (ACT) + causal mask (GPSIMD) for tile p."""
            sc_ps = ps.tile([128, 512], F32, tag="sc", name="sc_ps")
            y_sb, xT_sb = p["y_sb"], p["xT_sb"]
            for m in range(8):
                # batch-even: out partitions 0-63 (tile (0,0))
                nc.tensor.matmul(
                    sc_ps[0:64, 64 * m:64 * m + 64],
                    y_sb[:, 128 * m:128 * m + 64],
                    xT_sb[:, 128 * m:128 * m + 64],
                    start=True, stop=True,
                )
                # batch-odd: out partitions 64-127 (tile (0,64))
                nc.tensor.matmul(
                    sc_ps[64:128, 64 * m:64 * m + 64],
                    y_sb[:, 128 * m + 64:128 * m + 128],
                    xT_sb[:, 128 * m + 64:128 * m + 128],
                    start=True, stop=True,
                )
            E_raw = sb.tile([128, 512], BF16, tag="Eraw", name="E_raw")
            nc.scalar.activation(
                E_raw[:], sc_ps[:], mybir.ActivationFunctionType.Exp
            )
            E_sb = sb.tile([128, 512], BF16, tag="E", name="E_sb")
            nc.gpsimd.tensor_tensor(
                out=E_sb[:], in0=E_raw[:], in1=mk_sb[:],
                op=mybir.AluOpType.mult,
            )
            p["E_sb"] = E_sb

        def emit_C(p):
            """Row+col-tiled P4 + normalize + store for tile p."""
            o_ps = ps.tile([128, 1024], F32, tag="o", name="o_ps")
            E_sb, v_sb = p["E_sb"], p["v_sb"]
            for m in range(8):
                off = 512 * (m // 4) + 65 * (m % 4)
                # batch-even: rows 0-63 of E/v, out partitions 0-63
                nc.tensor.matmul(
                    o_ps[0:64, off:off + 65],
                    E_sb[0:64, 64 * m:64 * m + 64],
                    v_sb[0:64, 66 * m:66 * m + 65],
                    start=True, stop=True,
                )
                # batch-odd: rows 64-127, out partitions 64-127
                nc.tensor.matmul(
                    o_ps[64:128, off:off + 65],
                    E_sb[64:128, 64 * m:64 * m + 64],
                    v_sb[64:128, 66 * m:66 * m + 65],
                    start=True, stop=True,
                )
            opsv = o_ps[:].rearrange("p (B x) -> p B x", B=2)[:, :, 0:260]
            opsb = opsv.rearrange("p B (m z) -> p B m z", z=65)
            r_sb = sb.tile([128, 8], F32, tag="r", name="r_sb")
            r_v = r_sb[:].rearrange("p (B m) -> p B m", B=2)
            nc.vector.reciprocal(r_v.unsqueeze(3), opsb[:, :, :, 64:65])
            o_sb = sb.tile([128, 512], BF16, tag="o_sb", name="o_sb")
            nc.vector.tensor_tensor(
                out=o_sb[:].rearrange("p (B m t) -> p B m t", B=2, t=64),
                in0=opsb[:, :, :, 0:64],
                in1=r_v.unsqueeze(3).broadcast_to((128, 2, 4, 64)),
                op=mybir.AluOpType.mult,
            )
            nc.sync.dma_start(ov[p["st"]], o_sb[:])

        tiles = {}
        xT2_bufs = {0: emit_load(0)}
        for st in range(N_ST):
            if st % 2 == 0 and st // 2 + 1 < N_ST // 2:
                xT2_bufs[st // 2 + 1] = emit_load(st // 2 + 1)
            xT2 = xT2_bufs[st // 2]
            xT_view = xT2[:, (st % 2) * ROWS:(st % 2) * ROWS + ROWS]
            tiles[st] = emit_A(st, xT_view)
            if st >= 1:
                emit_B(tiles[st - 1])
            if st >= 2:
                emit_C(tiles[st - 2])
        emit_B(tiles[N_ST - 1])
        emit_C(tiles[N_ST - 2])
        emit_C(tiles[N_ST - 1])

    nc.compile()
    return nc


def _host_inputs(x, Wq, Wk, Wv):
    bf = ml_dtypes.bfloat16
    at = np.ascontiguousarray((Wk @ Wq.T * (C ** -0.5)).astype(bf))
    wv_bf = np.ascontiguousarray(Wv.astype(bf))
    tri = np.triu(np.ones((T, T), dtype=np.float32))  # [s, t]: 1 if s <= t
    mask = np.ascontiguousarray(
        np.tile(np.concatenate([tri, tri], axis=0), (1, 8)).astype(bf)
    )  # [128, 512]
    in_maps = []
    for c in range(N_CORES):
        shard = x[c * B_CORE:(c + 1) * B_CORE].reshape(B_CORE * T, C)
        xt = np.ascontiguousarray(shard.T).astype(bf)  # [128, 32768]
        in_maps.append({
            "xt": xt, "at": at, "wv": wv_bf, "mask": mask,
        })
    return in_maps


def _unshard(res_list):
    outs = []
    for r in res_list:
        o = np.asarray(r["o"], dtype=np.float32)  # [128, N_ST*512]
        # o[par*64+t, st*512 + m*64 + h] -> out[(st, m, par), t, h]
        o = o.reshape(2, 64, N_ST, 8, 64).transpose(2, 3, 0, 1, 4)
        outs.append(np.ascontiguousarray(o.reshape(B_CORE, T, H)))
    return np.concatenate(outs, axis=0)


def run(x, Wq, Wk, Wv, trace=False, **run_kwargs):
    from concourse import bass_utils

    if "nc" not in _cached:
        _cached["nc"] = _build_nc()
    nc = _cached["nc"]
    in_maps = _host_inputs(np.asarray(x), np.asarray(Wq),
                           np.asarray(Wk), np.asarray(Wv))
    res = bass_utils.run_bass_kernel_spmd(
        nc, in_maps, core_ids=list(range(N_CORES)), trace=trace, **run_kwargs
    )
    return _unshard(res.results), res


def kernel(x, Wq, Wk, Wv):
    out, _ = run(x, Wq, Wk, Wv, trace=False)
    return out
